# revision 6
# baseline (speedup 1.0000x reference)
"""LoRA multi-head attention on 8 TRN2 NeuronCores.

Sharding: data-parallel over batch (B=8 -> 1 batch element per core),
weights replicated, no collectives.

Host side: LoRA and the softmax scale are folded into the dense
weights (W'q = (Wq + Bq@Aq/16)/8, W'{k,v,o} = W + B@A/16) in fp32,
then transposed + cast bf16.  This is mathematically identical to the
reference and removes the whole LoRA path from the device.

Device side per core, all bf16 with fp32 PSUM accumulation:
  qT/kT = W'T.T @ xT per 128-row dout tile; v natural [n, dout]
  stored per-head as [v_h | 1] so PV also yields softmax denoms.
  Per head: S^T = kT_h.T qT_h -> exp (no max-sub; |s|=O(4)) -> PV;
  denominator row -> fast reciprocal (fp16) -> K=1 ones matmul
  broadcast -> normalize straight into attnT.
  out = attnT.T @ WoT + bo (bias via K=1 ones matmul).

The v projection runs kt-major in 4-chain waves gated on the
per-tile DMA arrivals of x/Wv, so compute starts ~1.5us into the
kernel instead of after the full weight load (doubles as HAM warmup).
"""

import sys

if "/opt/trn_rl_repo" not in sys.path:
    sys.path.insert(0, "/opt/trn_rl_repo")

import numpy as np
import ml_dtypes

BF16 = ml_dtypes.bfloat16

N = 1024  # tokens
D = 1024  # model dim
H = 16    # heads
HD = 64   # head dim
R = 16    # lora rank
P = 128   # partitions
F = 512   # psum free-dim tile
NCORES = 8
SCALING = 1.0 / 16.0  # lora alpha/rank
SCALE = HD ** -0.5

_CACHE = {}


def _build():
    import concourse.bacc as bacc
    import concourse.mybir as mybir
    import concourse.tile as tile

    f32 = mybir.dt.float32
    f16 = mybir.dt.float16
    bf16 = mybir.dt.bfloat16
    Exp = mybir.ActivationFunctionType.Exp

    nc = bacc.Bacc("TRN2", target_bir_lowering=False, debug=False)

    xT_e = nc.declare_dram_parameter("xT", [D, N], bf16, isOutput=False)
    wT_e = {
        nm: nc.declare_dram_parameter(nm, [D, D], bf16, isOutput=False)
        for nm in ("WqT", "WkT", "WvT", "WoT")
    }
    bo_e = nc.declare_dram_parameter("BoT", [1, D], bf16, isOutput=False)
    out_e = nc.declare_dram_parameter("out", [N, D], bf16, isOutput=True)

    with tile.TileContext(nc) as tc:
        with (
            tc.tile_pool(name="wpool", bufs=1) as wpool,
            tc.tile_pool(name="stage", bufs=2) as stage,
            tc.tile_pool(name="ps", bufs=1, space="PSUM") as ps,
        ):
            qs = [nc.sync, nc.scalar, nc.gpsimd]

            # ---- declare SBUF weight tiles; DMA them in pipeline order:
            # x[kt] and Wv[kt] interleaved first (v-phase is gated per
            # tile), then Wq/Wk (proj0), then Wo + bias. ----
            T = {}
            for nm in ("x", "Wv", "Wq", "Wk", "Wo"):
                T[nm] = [wpool.tile([P, D], bf16, tag=f"T_{nm}_{t}",
                                    name=f"T_{nm}_{t}") for t in range(8)]
            qi = 0

            def dma_tile(dst, src):
                nonlocal qi
                qs[qi % 3].dma_start(out=dst[:], in_=src)
                qi += 1

            for t in range(8):
                dma_tile(T["x"][t], xT_e[t * P:(t + 1) * P, :])
                dma_tile(T["Wv"][t], wT_e["WvT"][t * P:(t + 1) * P, :])
            for nm in ("Wq", "Wk", "Wo"):
                for t in range(8):
                    dma_tile(T[nm][t], wT_e[nm + "T"][t * P:(t + 1) * P, :])
            bot = wpool.tile([1, D], bf16, tag="bot")
            dma_tile(bot, bo_e[:, :])

            onesh = wpool.tile([1, HD], f16, tag="onesh")
            nc.vector.memset(onesh[:], 1.0)
            onesb = wpool.tile([1, P], bf16, tag="onesb")
            nc.vector.memset(onesb[:], 1.0)

            # ---- tiny HAM kick, gated on the first x tile ----
            wps = ps.tile([P, F], f32, tag="tpsum", bufs=1)
            for _ in range(6):
                nc.tensor.matmul(wps[:, 0:256], T["x"][0][:, 0:P],
                                 T["x"][0][:, 0:256], start=True, stop=True)

            # ---- q/k projection generator (dense only; lora folded) ----
            qks = {}

            def proj_gen(dt):
                qk = {}
                for nm, wnm in (("q", "Wq"), ("k", "Wk")):
                    dst = wpool.tile([P, D], bf16, tag=f"{nm}T",
                                     bufs=3, name=f"{nm}T_{dt}")
                    qk[nm] = dst
                    for nh in range(2):
                        ns = slice(nh * F, (nh + 1) * F)
                        pq = ps.tile([P, F], f32, tag="projpsum", bufs=1)
                        for kt in range(8):
                            nc.tensor.matmul(
                                pq[:], T[wnm][kt][:, dt * P:(dt + 1) * P],
                                T["x"][kt][:, ns],
                                start=(kt == 0), stop=(kt == 7))
                            yield
                        nc.vector.tensor_copy(dst[:, ns], pq[:])
                        yield
                qks[dt] = qk

            # ---- v natural, per-head layout [v_h | 1]; kt-major waves
            # of 4 chains so compute is gated on per-tile DMA arrivals.
            # proj(0) is woven into the later waves. ----
            VW = H * (HD + 1)  # 1040
            v_sb = [wpool.tile([P, VW], bf16, tag=f"v_{t}",
                               name=f"v_{t}") for t in range(8)]
            g0 = proj_gen(0)
            for wv in range(4):
                nts = (2 * wv, 2 * wv + 1)
                pv = {nt: ps.tile([P, 2 * F], f32, tag="spair", bufs=2,
                                  name=f"pv_{nt}")
                      for nt in nts}
                for kt in range(8):
                    for nt in nts:
                        for dh in range(2):
                            nc.tensor.matmul(
                                pv[nt][:, dh * F:(dh + 1) * F],
                                T["x"][kt][:, nt * P:(nt + 1) * P],
                                T["Wv"][kt][:, dh * F:(dh + 1) * F],
                                start=(kt == 0), stop=(kt == 7))
                    if wv >= 2:
                        for _ in range(3):
                            next(g0, None)
                for nt in nts:
                    vr = v_sb[nt][:].rearrange("p (h c) -> p h c", c=HD + 1)
                    for dh in range(2):
                        pvr = pv[nt][:, dh * F:(dh + 1) * F].rearrange(
                            "p (h c) -> p h c", c=HD)
                        nc.vector.tensor_copy(
                            vr[:, dh * 8:(dh + 1) * 8, 0:HD], pvr[:])
                        if wv >= 2:
                            for _ in range(2):
                                next(g0, None)
                    nc.vector.memset(vr[:, :, HD:HD + 1], 1.0)
            for _ in g0:
                pass

            # ---- per dout-tile: its 2 heads' attention, with the NEXT
            # tile's projection matmuls woven in so the PE stays dense
            # while ACT runs the exps. ----
            attnT = [wpool.tile([P, D], bf16, tag=f"attnT_{t}",
                                name=f"attnT_{t}") for t in range(8)]
            for dt in range(8):
                g = proj_gen(dt + 1) if dt < 7 else iter(())
                h0 = 2 * dt
                qt = qks[dt]["q"]
                ktt = qks[dt]["k"]
                for nh in range(2):
                    ns = slice(nh * F, (nh + 1) * F)
                    po = {}
                    for h in (h0, h0 + 1):
                        po[h] = ps.tile([HD + 1, F], f32, tag="pvpsum",
                                        bufs=2, name=f"po_{h}_{nh}")
                    for mt in range(8):
                        spair = ps.tile([P, 2 * F], f32, tag="spair",
                                        bufs=2)
                        for hi, h in enumerate((h0, h0 + 1)):
                            ro = (h % 2) * HD
                            m0 = mt * P
                            nc.tensor.matmul(
                                spair[:, hi * F:(hi + 1) * F],
                                ktt[ro:ro + HD, m0:m0 + P],
                                qt[ro:ro + HD, ns], start=True, stop=True)
                        pte = stage.tile([P, 2 * F], bf16, tag="pt", bufs=3)
                        nc.scalar.activation(pte[:], spair[:], Exp)
                        for hi, h in enumerate((h0, h0 + 1)):
                            nc.tensor.matmul(
                                po[h][:],
                                v_sb[mt][:, h * (HD + 1):(h + 1) * (HD + 1)],
                                pte[:, hi * F:(hi + 1) * F],
                                start=(mt == 0), stop=(mt == 7))
                        for _ in range(2):
                            next(g, None)
                    for h in (h0, h0 + 1):
                        ro = (h % 2) * HD
                        oah = stage.tile([HD + 1, F], f32, tag="oah", bufs=3)
                        nc.vector.tensor_copy(oah[:], po[h][:])
                        pr32 = stage.tile([1, F], f32, tag="pr32", bufs=3)
                        nc.vector.reciprocal(pr32[:], oah[HD:HD + 1, :])
                        pr = stage.tile([1, F], f16, tag="pr", bufs=3)
                        nc.vector.tensor_copy(pr[:], pr32[:])
                        pb = ps.tile([HD, F], f32, tag="tpsum", bufs=1)
                        nc.tensor.matmul(pb[:], onesh[0:1, :], pr[:],
                                         start=True, stop=True)
                        nc.vector.tensor_mul(attnT[dt][ro:ro + HD, ns],
                                             oah[0:HD, :], pb[:])
                        for _ in range(2):
                            next(g, None)
                for _ in g:
                    pass

            # ---- output projection (+ bias via K=1 ones matmul) ----
            for nt in range(8):
                for dh in range(2):
                    ds = slice(dh * F, (dh + 1) * F)
                    pf = ps.tile([P, F], f32, tag="spair", bufs=2)
                    for kt in range(8):
                        nc.tensor.matmul(pf[:],
                                         attnT[kt][:, nt * P:(nt + 1) * P],
                                         T["Wo"][kt][:, ds],
                                         start=(kt == 0), stop=False)
                    nc.tensor.matmul(pf[:], onesb[0:1, :], bot[0:1, ds],
                                     start=False, stop=True)
                    osb = stage.tile([P, F], bf16, tag="osb")
                    nc.vector.tensor_copy(osb[:], pf[:])
                    nc.sync.dma_start(out=out_e[nt * P:(nt + 1) * P, ds],
                                      in_=osb[:])
    nc.compile()
    return nc


def _get_nc():
    if "nc" not in _CACHE:
        _CACHE["nc"] = _build()
    return _CACHE["nc"]


def _prep_shared(inputs):
    def fold(w, a, b, scl=1.0):
        w = np.asarray(w, np.float32)
        a = np.asarray(a, np.float32)
        b = np.asarray(b, np.float32)
        eff = (w + (b @ a) * SCALING) * scl
        return np.ascontiguousarray(eff.T.astype(BF16))

    shared = {
        "WqT": fold(inputs["Wq"], inputs["Aq"], inputs["Bq"], SCALE),
        "WkT": fold(inputs["Wk"], inputs["Ak"], inputs["Bk"]),
        "WvT": fold(inputs["Wv"], inputs["Av"], inputs["Bv"]),
        "WoT": fold(inputs["Wo"], inputs["Ao"], inputs["Bo"]),
        "BoT": np.ascontiguousarray(
            np.asarray(inputs["bo"], np.float32).reshape(1, D).astype(BF16)),
    }
    return shared


def kernel(**inputs):
    from concourse import bass_utils

    nc = _get_nc()
    shared = _prep_shared(inputs)
    x = np.asarray(inputs["x"], np.float32)
    in_maps = []
    for i in range(NCORES):
        m = dict(shared)
        m["xT"] = np.ascontiguousarray(x[i].T.astype(BF16))
        in_maps.append(m)
    res = bass_utils.run_bass_kernel_spmd(nc, in_maps,
                                          core_ids=list(range(NCORES)))
    return np.stack([np.asarray(res.results[i]["out"]).astype(np.float32)
                     for i in range(NCORES)], axis=0)


# revision 8
# speedup vs baseline: 1.4667x; 1.4667x over previous
"""LoRA multi-head attention on 8 TRN2 NeuronCores.

Sharding: data-parallel over batch (B=8 -> 1 batch element per core),
weights replicated, no collectives.

Host side: LoRA and the softmax scale are folded into the dense
weights (W'q = (Wq + Bq@Aq/16)/8, W'{k,v,o} = W + B@A/16) in fp32,
then transposed + cast bf16.  This is mathematically identical to the
reference and removes the whole LoRA path from the device.

Device side per core, all bf16 with fp32 PSUM accumulation:
  qT/kT = W'T.T @ xT per 128-row dout tile; v natural [n, dout]
  stored per-head as [v_h | 1] so PV also yields softmax denoms.
  Per head: S^T = kT_h.T qT_h -> exp (no max-sub; |s|=O(4)) -> PV;
  denominator row -> fast reciprocal (fp16) -> K=1 ones matmul
  broadcast -> normalize straight into attnT.
  out = attnT.T @ WoT + bo (bias via K=1 ones matmul).

The v projection runs kt-major in 4-chain waves gated on the
per-tile DMA arrivals of x/Wv, so compute starts ~1.5us into the
kernel instead of after the full weight load (doubles as HAM warmup).
"""

import sys

if "/opt/trn_rl_repo" not in sys.path:
    sys.path.insert(0, "/opt/trn_rl_repo")

import numpy as np
import ml_dtypes

BF16 = ml_dtypes.bfloat16

N = 1024  # tokens
D = 1024  # model dim
H = 16    # heads
HD = 64   # head dim
R = 16    # lora rank
P = 128   # partitions
F = 512   # psum free-dim tile
NCORES = 8
SCALING = 1.0 / 16.0  # lora alpha/rank
SCALE = HD ** -0.5

_CACHE = {}


def _build():
    import concourse.bacc as bacc
    import concourse.mybir as mybir
    import concourse.tile as tile

    f32 = mybir.dt.float32
    f16 = mybir.dt.float16
    bf16 = mybir.dt.bfloat16
    Exp = mybir.ActivationFunctionType.Exp

    nc = bacc.Bacc("TRN2", target_bir_lowering=False, debug=False)

    xT_e = nc.declare_dram_parameter("xT", [D, N], bf16, isOutput=False)
    wT_e = {
        nm: nc.declare_dram_parameter(nm, [D, D], bf16, isOutput=False)
        for nm in ("WqT", "WkT", "WvT", "WoT")
    }
    bo_e = nc.declare_dram_parameter("BoT", [1, D], bf16, isOutput=False)
    out_e = nc.declare_dram_parameter("out", [N, D], bf16, isOutput=True)

    with tile.TileContext(nc) as tc:
        with (
            tc.tile_pool(name="wpool", bufs=1) as wpool,
            tc.tile_pool(name="stage", bufs=2) as stage,
            tc.tile_pool(name="ps", bufs=1, space="PSUM") as ps,
        ):
            qs = [nc.sync, nc.scalar, nc.gpsimd]

            # ---- declare SBUF weight tiles; DMA them in pipeline order:
            # x[kt] and Wv[kt] interleaved first (v-phase is gated per
            # tile), then Wq/Wk (proj0), then Wo + bias. ----
            T = {}
            for nm in ("x", "Wv", "Wq", "Wk", "Wo"):
                T[nm] = [wpool.tile([P, D], bf16, tag=f"T_{nm}_{t}",
                                    name=f"T_{nm}_{t}") for t in range(8)]
            qi = 0

            def dma_tile(dst, src):
                nonlocal qi
                qs[qi % 3].dma_start(out=dst[:], in_=src)
                qi += 1

            for t in range(8):
                dma_tile(T["x"][t], xT_e[t * P:(t + 1) * P, :])
                dma_tile(T["Wv"][t], wT_e["WvT"][t * P:(t + 1) * P, :])
            for nm in ("Wq", "Wk", "Wo"):
                for t in range(8):
                    dma_tile(T[nm][t], wT_e[nm + "T"][t * P:(t + 1) * P, :])
            bot = wpool.tile([1, D], bf16, tag="bot")
            dma_tile(bot, bo_e[:, :])

            onesh = wpool.tile([1, HD], f16, tag="onesh")
            nc.vector.memset(onesh[:], 1.0)
            onesb = wpool.tile([1, P], bf16, tag="onesb")
            nc.vector.memset(onesb[:], 1.0)

            # ---- tiny HAM kick, gated on the first x tile ----
            wps = ps.tile([P, F], f32, tag="tpsum", bufs=1)
            for _ in range(6):
                nc.tensor.matmul(wps[:, 0:256], T["x"][0][:, 0:P],
                                 T["x"][0][:, 0:256], start=True, stop=True)

            # ---- q/k projection generator (dense only; lora folded) ----
            qks = {}

            def proj_gen(dt):
                qk = {}
                for nm, wnm in (("q", "Wq"), ("k", "Wk")):
                    dst = wpool.tile([P, D], bf16, tag=f"{nm}T",
                                     bufs=3, name=f"{nm}T_{dt}")
                    qk[nm] = dst
                    for nh in range(2):
                        ns = slice(nh * F, (nh + 1) * F)
                        pq = ps.tile([P, F], f32, tag="projpsum", bufs=1)
                        for kt in range(8):
                            nc.tensor.matmul(
                                pq[:], T[wnm][kt][:, dt * P:(dt + 1) * P],
                                T["x"][kt][:, ns],
                                start=(kt == 0), stop=(kt == 7))
                            yield
                        nc.vector.tensor_copy(dst[:, ns], pq[:])
                        yield
                qks[dt] = qk

            # ---- v natural, per-head layout [v_h | 1]; kt-major waves
            # of 4 chains so compute is gated on per-tile DMA arrivals.
            # proj(0) is woven into the later waves. ----
            VW = H * (HD + 1)  # 1040
            v_sb = [wpool.tile([P, VW], bf16, tag=f"v_{t}",
                               name=f"v_{t}") for t in range(8)]
            g0 = proj_gen(0)
            for wv in range(4):
                nts = (2 * wv, 2 * wv + 1)
                pv = {nt: ps.tile([P, 2 * F], f32, tag="spair", bufs=2,
                                  name=f"pv_{nt}")
                      for nt in nts}
                for kt in range(8):
                    for nt in nts:
                        for dh in range(2):
                            nc.tensor.matmul(
                                pv[nt][:, dh * F:(dh + 1) * F],
                                T["x"][kt][:, nt * P:(nt + 1) * P],
                                T["Wv"][kt][:, dh * F:(dh + 1) * F],
                                start=(kt == 0), stop=(kt == 7))
                    if wv >= 2:
                        for _ in range(3):
                            next(g0, None)
                for nt in nts:
                    vr = v_sb[nt][:].rearrange("p (h c) -> p h c", c=HD + 1)
                    for dh in range(2):
                        pvr = pv[nt][:, dh * F:(dh + 1) * F].rearrange(
                            "p (h c) -> p h c", c=HD)
                        nc.vector.tensor_copy(
                            vr[:, dh * 8:(dh + 1) * 8, 0:HD], pvr[:])
                        if wv >= 2:
                            for _ in range(2):
                                next(g0, None)
                    nc.vector.memset(vr[:, :, HD:HD + 1], 1.0)
            for _ in g0:
                pass

            # ---- per dout-tile: its 2 heads' attention, with the NEXT
            # tile's projection matmuls woven in so the PE stays dense
            # while ACT runs the exps. ----
            attnT = [wpool.tile([P, D], bf16, tag=f"attnT_{t}",
                                name=f"attnT_{t}") for t in range(8)]
            for dt in range(8):
                g = proj_gen(dt + 1) if dt < 7 else iter(())
                h0 = 2 * dt
                qt = qks[dt]["q"]
                ktt = qks[dt]["k"]
                for nh in range(2):
                    ns = slice(nh * F, (nh + 1) * F)
                    po = {}
                    for h in (h0, h0 + 1):
                        po[h] = ps.tile([HD + 1, F], f32, tag="pvpsum",
                                        bufs=2, name=f"po_{h}_{nh}")
                    for mt in range(8):
                        spair = ps.tile([P, 2 * F], f32, tag="spair",
                                        bufs=2)
                        for hi, h in enumerate((h0, h0 + 1)):
                            ro = (h % 2) * HD
                            m0 = mt * P
                            nc.tensor.matmul(
                                spair[:, hi * F:(hi + 1) * F],
                                ktt[ro:ro + HD, m0:m0 + P],
                                qt[ro:ro + HD, ns], start=True, stop=True)
                        pte = stage.tile([P, 2 * F], bf16, tag="pt", bufs=3)
                        nc.scalar.activation(pte[:], spair[:], Exp)
                        for hi, h in enumerate((h0, h0 + 1)):
                            nc.tensor.matmul(
                                po[h][:],
                                v_sb[mt][:, h * (HD + 1):(h + 1) * (HD + 1)],
                                pte[:, hi * F:(hi + 1) * F],
                                start=(mt == 0), stop=(mt == 7))
                        for _ in range(2):
                            next(g, None)
                    for h in (h0, h0 + 1):
                        ro = (h % 2) * HD
                        oah = stage.tile([HD + 1, F], f32, tag="oah", bufs=3)
                        nc.vector.tensor_copy(oah[:], po[h][:])
                        dn = stage.tile([1, F], f16, tag="dn", bufs=3)
                        nc.vector.tensor_copy(dn[:], oah[HD:HD + 1, :])
                        pb = ps.tile([HD, F], f32, tag="tpsum", bufs=1)
                        nc.tensor.matmul(pb[:], onesh[0:1, :], dn[:],
                                         start=True, stop=True)
                        for _ in range(2):
                            next(g, None)
                        pbs = stage.tile([HD, F], f32, tag="pbs", bufs=3)
                        nc.vector.reciprocal_approx_fast(pbs[:], pb[:])
                        nc.vector.tensor_mul(attnT[dt][ro:ro + HD, ns],
                                             oah[0:HD, :], pbs[:])
                        for _ in range(2):
                            next(g, None)
                for _ in g:
                    pass

            # ---- output projection (+ bias via K=1 ones matmul) ----
            for nt in range(8):
                for dh in range(2):
                    ds = slice(dh * F, (dh + 1) * F)
                    pf = ps.tile([P, F], f32, tag="spair", bufs=2)
                    for kt in range(8):
                        nc.tensor.matmul(pf[:],
                                         attnT[kt][:, nt * P:(nt + 1) * P],
                                         T["Wo"][kt][:, ds],
                                         start=(kt == 0), stop=False)
                    nc.tensor.matmul(pf[:], onesb[0:1, :], bot[0:1, ds],
                                     start=False, stop=True)
                    osb = stage.tile([P, F], bf16, tag="osb")
                    nc.vector.tensor_copy(osb[:], pf[:])
                    nc.sync.dma_start(out=out_e[nt * P:(nt + 1) * P, ds],
                                      in_=osb[:])
    nc.compile()
    return nc


def _get_nc():
    if "nc" not in _CACHE:
        _CACHE["nc"] = _build()
    return _CACHE["nc"]


def _prep_shared(inputs):
    def fold(w, a, b, scl=1.0):
        w = np.asarray(w, np.float32)
        a = np.asarray(a, np.float32)
        b = np.asarray(b, np.float32)
        eff = (w + (b @ a) * SCALING) * scl
        return np.ascontiguousarray(eff.T.astype(BF16))

    shared = {
        "WqT": fold(inputs["Wq"], inputs["Aq"], inputs["Bq"], SCALE),
        "WkT": fold(inputs["Wk"], inputs["Ak"], inputs["Bk"]),
        "WvT": fold(inputs["Wv"], inputs["Av"], inputs["Bv"]),
        "WoT": fold(inputs["Wo"], inputs["Ao"], inputs["Bo"]),
        "BoT": np.ascontiguousarray(
            np.asarray(inputs["bo"], np.float32).reshape(1, D).astype(BF16)),
    }
    return shared


def kernel(**inputs):
    from concourse import bass_utils

    nc = _get_nc()
    shared = _prep_shared(inputs)
    x = np.asarray(inputs["x"], np.float32)
    in_maps = []
    for i in range(NCORES):
        m = dict(shared)
        m["xT"] = np.ascontiguousarray(x[i].T.astype(BF16))
        in_maps.append(m)
    res = bass_utils.run_bass_kernel_spmd(nc, in_maps,
                                          core_ids=list(range(NCORES)))
    return np.stack([np.asarray(res.results[i]["out"]).astype(np.float32)
                     for i in range(NCORES)], axis=0)


# revision 12
# speedup vs baseline: 1.4817x; 1.0102x over previous
"""LoRA multi-head attention on 8 TRN2 NeuronCores.

Sharding: data-parallel over batch (B=8 -> 1 batch element per core),
weights replicated, no collectives.

Host side: LoRA and the softmax scale are folded into the dense
weights (W'q = (Wq + Bq@Aq/16)/8, W'{k,v,o} = W + B@A/16) in fp32,
then transposed + cast bf16.  Mathematically identical to the
reference; removes the whole LoRA path from the device.

Device side per core, all bf16 with fp32 PSUM accumulation:
  qT/kT = W'T.T @ xT per 128-row dout tile; v natural [n, dout]
  stored per-head as [v_h | 1] so PV also yields softmax denoms.
  Per head pair: S^T via row-tiled (64x128) concurrent matmuls ->
  exp on ACT -> PV -> denominator row -> f16 K=1 ones-matmul
  broadcast -> fast reciprocal -> normalize straight into attnT.
  out = attnT.T @ WoT + bo (bias via K=1 ones matmul).

Scheduling: x/Wv are DMAed as column-half tiles interleaved per kt so
the kt-major v-projection waves start ~2us into the kernel (doubling
as HAM warmup); q/k projections for tile dt+1 are woven into tile
dt's attention groups; the first 6 output-projection chains are woven
into dt=7 (whose weave has no next projection).
"""

import sys

if "/opt/trn_rl_repo" not in sys.path:
    sys.path.insert(0, "/opt/trn_rl_repo")

import numpy as np
import ml_dtypes

BF16 = ml_dtypes.bfloat16

N = 1024  # tokens
D = 1024  # model dim
H = 16    # heads
HD = 64   # head dim
P = 128   # partitions
F = 512   # psum free-dim tile
NCORES = 8
SCALING = 1.0 / 16.0  # lora alpha/rank
SCALE = HD ** -0.5

_CACHE = {}


def _build():
    import concourse.bacc as bacc
    import concourse.mybir as mybir
    import concourse.tile as tile

    f32 = mybir.dt.float32
    f16 = mybir.dt.float16
    bf16 = mybir.dt.bfloat16
    Exp = mybir.ActivationFunctionType.Exp
    Copy = mybir.ActivationFunctionType.Copy

    nc = bacc.Bacc("TRN2", target_bir_lowering=False, debug=False)

    xT_e = nc.declare_dram_parameter("xT", [D, N], bf16, isOutput=False)
    wT_e = {
        nm: nc.declare_dram_parameter(nm, [D, D], bf16, isOutput=False)
        for nm in ("WqT", "WkT", "WvT", "WoT")
    }
    bo_e = nc.declare_dram_parameter("BoT", [1, D], bf16, isOutput=False)
    out_e = nc.declare_dram_parameter("out", [N, D], bf16, isOutput=True)

    with tile.TileContext(nc) as tc:
        with (
            tc.tile_pool(name="wpool", bufs=1) as wpool,
            tc.tile_pool(name="stage", bufs=2) as stage,
            tc.tile_pool(name="ps", bufs=1, space="PSUM") as ps,
        ):
            qs = [nc.sync, nc.scalar, nc.gpsimd]
            qi = 0

            def dma_tile(dst, src):
                nonlocal qi
                qs[qi % 3].dma_start(out=dst[:], in_=src)
                qi += 1

            # x and Wv live as column-half tiles [P, F] so the first
            # v-wave's inputs can be DMAed first.
            xh = [[wpool.tile([P, F], bf16, tag=f"x_{t}_{h}",
                              name=f"x_{t}_{h}") for h in range(2)]
                  for t in range(8)]
            vh = [[wpool.tile([P, F], bf16, tag=f"Wv_{t}_{h}",
                              name=f"Wv_{t}_{h}") for h in range(2)]
                  for t in range(8)]
            T = {}
            for nm in ("Wq", "Wk", "Wo"):
                T[nm] = [wpool.tile([P, D], bf16, tag=f"T_{nm}_{t}",
                                    name=f"T_{nm}_{t}") for t in range(8)]
            for t in range(8):
                dma_tile(xh[t][0], xT_e[t * P:(t + 1) * P, 0:F])
                dma_tile(vh[t][0], wT_e["WvT"][t * P:(t + 1) * P, 0:F])
            for t in range(8):
                dma_tile(xh[t][1], xT_e[t * P:(t + 1) * P, F:2 * F])
                dma_tile(vh[t][1], wT_e["WvT"][t * P:(t + 1) * P, F:2 * F])
            for nm in ("Wq", "Wk", "Wo"):
                for t in range(8):
                    dma_tile(T[nm][t], wT_e[nm + "T"][t * P:(t + 1) * P, :])
            bot = wpool.tile([1, D], bf16, tag="bot")
            dma_tile(bot, bo_e[:, :])

            def xcol(t, c0, c1):  # x[t] columns [c0, c1) across halves
                h = c0 // F
                return xh[t][h][:, c0 - h * F:c1 - h * F]

            onesh = wpool.tile([1, HD], f16, tag="onesh")
            nc.vector.memset(onesh[:], 1.0)
            onesb = wpool.tile([1, P], bf16, tag="onesb")
            nc.vector.memset(onesb[:], 1.0)

            # ---- tiny HAM kick, gated on the first x half-tile ----
            wps = ps.tile([P, F], f32, tag="projpsum", bufs=2)
            for _ in range(6):
                nc.tensor.matmul(wps[:, 0:256], xh[0][0][:, 0:P],
                                 xh[0][0][:, 0:256], start=True, stop=True)

            # ---- q/k projection generator (dense only; lora folded) ----
            qks = {}

            def proj_gen(dt):
                qk = {}
                for nm, wnm in (("q", "Wq"), ("k", "Wk")):
                    dst = wpool.tile([P, D], bf16, tag=f"{nm}T",
                                     bufs=3, name=f"{nm}T_{dt}")
                    qk[nm] = dst
                    for nh in range(2):
                        ns = slice(nh * F, (nh + 1) * F)
                        pq = ps.tile([P, F], f32, tag="projpsum", bufs=2)
                        for kt in range(8):
                            nc.tensor.matmul(
                                pq[:], T[wnm][kt][:, dt * P:(dt + 1) * P],
                                xh[kt][nh][:],
                                start=(kt == 0), stop=(kt == 7))
                            yield
                        nc.vector.tensor_copy(dst[:, ns], pq[:])
                        yield
                qks[dt] = qk

            # ---- v natural, per-head layout [v_h | 1]; kt-major waves
            # gated on the half-tile DMA arrivals. proj(0) woven in
            # once Wq/Wk start landing. ----
            VW = H * (HD + 1)  # 1040
            v_sb = [wpool.tile([P, VW], bf16, tag=f"v_{t}",
                               name=f"v_{t}") for t in range(8)]
            g0 = proj_gen(0)
            waves = [[(nt, 0) for nt in range(4)],
                     [(nt, 1) for nt in range(4)],
                     [(nt, 0) for nt in range(4, 8)],
                     [(nt, 1) for nt in range(4, 8)]]
            for wv, chains in enumerate(waves):
                pv = {}
                for ci, (nt, dh) in enumerate(chains):
                    if ci % 2 == 0:
                        pvt = ps.tile([P, 2 * F], f32, tag="spair", bufs=2,
                                      name=f"pvt_{wv}_{ci}")
                    pv[(nt, dh)] = pvt[:, (ci % 2) * F:(ci % 2 + 1) * F]
                for kt in range(8):
                    for (nt, dh) in chains:
                        nc.tensor.matmul(
                            pv[(nt, dh)], xcol(kt, nt * P, (nt + 1) * P),
                            vh[kt][dh][:],
                            start=(kt == 0), stop=(kt == 7))
                    if wv >= 2:
                        for _ in range(3):
                            next(g0, None)
                for (nt, dh) in chains:
                    vr = v_sb[nt][:].rearrange("p (h c) -> p h c", c=HD + 1)
                    pvr = pv[(nt, dh)].rearrange("p (h c) -> p h c", c=HD)
                    nc.vector.tensor_copy(
                        vr[:, dh * 8:(dh + 1) * 8, 0:HD], pvr[:])
                    if wv >= 2:
                        for _ in range(2):
                            next(g0, None)
                    if dh == 1:
                        nc.vector.memset(vr[:, :, HD:HD + 1], 1.0)
            for _ in g0:
                pass

            # ---- output projection chain helper (kt-accumulating,
            # + bias via K=1 ones matmul, drain via DVE/ACT) ----
            attnT = [wpool.tile([P, D], bf16, tag=f"attnT_{t}",
                                name=f"attnT_{t}") for t in range(8)]

            def oproj_head(nt, dh, upto):
                # kt 0..upto-1 accumulation; returns psum handle via
                # the last yielded value
                pf = ps.tile([P, F], f32, tag="projpsum", bufs=2,
                             name=f"pf_{nt}_{dh}")
                for kt in range(upto):
                    nc.tensor.matmul(pf[:],
                                     attnT[kt][:, nt * P:(nt + 1) * P],
                                     T["Wo"][kt][:, dh * F:(dh + 1) * F],
                                     start=(kt == 0), stop=False)
                    yield pf

            def oproj_tail(pf, nt, dh, upto, on_act):
                ds = slice(dh * F, (dh + 1) * F)
                for kt in range(upto, 8):
                    nc.tensor.matmul(pf[:],
                                     attnT[kt][:, nt * P:(nt + 1) * P],
                                     T["Wo"][kt][:, ds],
                                     start=False, stop=False)
                nc.tensor.matmul(pf[:], onesb[0:1, :], bot[0:1, ds],
                                 start=False, stop=True)
                osb = stage.tile([P, F], bf16, tag="osb", bufs=3)
                if on_act:
                    nc.scalar.activation(osb[:], pf[:], Copy)
                else:
                    nc.vector.tensor_copy(osb[:], pf[:])
                nc.sync.dma_start(out=out_e[nt * P:(nt + 1) * P, ds],
                                  in_=osb[:])

            def oproj_chain(nt, dh, on_act):
                pf = None
                for pf in oproj_head(nt, dh, 8):
                    yield
                oproj_tail(pf, nt, dh, 8, on_act)
                yield

            # ---- attention per dout-tile; next tile's projections are
            # woven into the groups (for dt=7: the first o-proj chains)
            woven = [(0, 0), (0, 1)]
            for dt in range(8):
                if dt < 7:
                    g = proj_gen(dt + 1)
                else:
                    # weave only kt<=6 of 2 o-proj chains (attnT[7] is
                    # written during this dt; projpsum has 2 buffers)
                    def _dt7_gen():
                        for nt, dh in woven:
                            pfh = None
                            for pfh in oproj_head(nt, dh, 7):
                                yield
                            _pf_held.append((pfh, nt, dh))
                    _pf_held = []
                    g = _dt7_gen()
                h0 = 2 * dt
                qt = qks[dt]["q"]
                ktt = qks[dt]["k"]
                for nh in range(2):
                    ns = slice(nh * F, (nh + 1) * F)
                    po = {}
                    for h in (h0, h0 + 1):
                        po[h] = ps.tile([HD + 1, F], f32, tag="pvpsum",
                                        bufs=2, name=f"po_{h}_{nh}")
                    for mt in range(8):
                        spair = ps.tile([P, 2 * F], f32, tag="spair",
                                        bufs=2)
                        for hi, h in enumerate((h0, h0 + 1)):
                            ro = (h % 2) * HD
                            m0 = mt * P
                            nc.tensor.matmul(
                                spair[:, hi * F:(hi + 1) * F],
                                ktt[ro:ro + HD, m0:m0 + P],
                                qt[ro:ro + HD, ns], start=True, stop=True)
                        pte = stage.tile([P, 2 * F], bf16, tag="pt", bufs=3)
                        nc.scalar.activation(pte[:], spair[:], Exp)
                        for hi, h in enumerate((h0, h0 + 1)):
                            nc.tensor.matmul(
                                po[h][:],
                                v_sb[mt][:, h * (HD + 1):(h + 1) * (HD + 1)],
                                pte[:, hi * F:(hi + 1) * F],
                                start=(mt == 0), stop=(mt == 7))
                        for _ in range(2 if mt < 4 else 1):
                            next(g, None)
                    for h in (h0, h0 + 1):
                        ro = (h % 2) * HD
                        oah = stage.tile([HD + 1, F], f32, tag="oah", bufs=3)
                        nc.vector.tensor_copy(oah[:], po[h][:])
                        dn = stage.tile([1, F], f16, tag="dn", bufs=3)
                        nc.vector.tensor_copy(dn[:], oah[HD:HD + 1, :])
                        pb = ps.tile([HD, F], f32, tag="pvpsum", bufs=2)
                        nc.tensor.matmul(pb[:], onesh[0:1, :], dn[:],
                                         start=True, stop=True)
                        for _ in range(2):
                            next(g, None)
                        pbs = stage.tile([HD, F], f32, tag="pbs", bufs=3)
                        nc.vector.reciprocal_approx_fast(pbs[:], pb[:])
                        nc.vector.tensor_mul(attnT[dt][ro:ro + HD, ns],
                                             oah[0:HD, :], pbs[:])
                for _ in g:
                    pass

            # ---- finish woven o-proj chains, then the rest ----
            for i, (pfh, nt, dh) in enumerate(_pf_held):
                oproj_tail(pfh, nt, dh, 7, on_act=(i % 2 == 1))
            rest = [(nt, dh) for nt in range(8) for dh in range(2)
                    if (nt, dh) not in woven]
            for i, (nt, dh) in enumerate(rest):
                for _ in oproj_chain(nt, dh, on_act=(i % 2 == 1)):
                    pass
    nc.compile()
    return nc


def _get_nc():
    if "nc" not in _CACHE:
        _CACHE["nc"] = _build()
    return _CACHE["nc"]


def _prep_shared(inputs):
    def fold(w, a, b, scl=1.0):
        w = np.asarray(w, np.float32)
        a = np.asarray(a, np.float32)
        b = np.asarray(b, np.float32)
        eff = (w + (b @ a) * SCALING) * scl
        return np.ascontiguousarray(eff.T.astype(BF16))

    shared = {
        "WqT": fold(inputs["Wq"], inputs["Aq"], inputs["Bq"], SCALE),
        "WkT": fold(inputs["Wk"], inputs["Ak"], inputs["Bk"]),
        "WvT": fold(inputs["Wv"], inputs["Av"], inputs["Bv"]),
        "WoT": fold(inputs["Wo"], inputs["Ao"], inputs["Bo"]),
        "BoT": np.ascontiguousarray(
            np.asarray(inputs["bo"], np.float32).reshape(1, D).astype(BF16)),
    }
    return shared


def kernel(**inputs):
    from concourse import bass_utils

    nc = _get_nc()
    shared = _prep_shared(inputs)
    x = np.asarray(inputs["x"], np.float32)
    in_maps = []
    for i in range(NCORES):
        m = dict(shared)
        m["xT"] = np.ascontiguousarray(x[i].T.astype(BF16))
        in_maps.append(m)
    res = bass_utils.run_bass_kernel_spmd(nc, in_maps,
                                          core_ids=list(range(NCORES)))
    return np.stack([np.asarray(res.results[i]["out"]).astype(np.float32)
                     for i in range(NCORES)], axis=0)


# revision 13
# speedup vs baseline: 1.5699x; 1.0595x over previous
"""LoRA multi-head attention on 8 TRN2 NeuronCores.

Sharding: data-parallel over batch (B=8 -> 1 batch element per core),
weights replicated, no collectives.

Host side: LoRA and the softmax scale are folded into the dense
weights (W'q = (Wq + Bq@Aq/16)/8, W'{k,v,o} = W + B@A/16) in fp32,
then transposed + cast bf16.  Mathematically identical to the
reference; removes the whole LoRA path from the device.

Device side per core, all bf16 with fp32 PSUM accumulation:
  qT/kT = W'T.T @ xT per 128-row dout tile; v natural [n, dout]
  stored per-head as [v_h | 1] so PV also yields softmax denoms.
  Attention runs in steps of two m-tiles: S^T for both heads via
  row-tiled (64x128) concurrent matmuls -> exp on ACT -> PV lagged
  one step (reads the previous step's probabilities, so it never
  waits on ACT) -> denominator row -> f16 K=1 ones-matmul broadcast
  -> fast reciprocal -> normalize straight into attnT.
  out = attnT.T @ WoT (+ bo via K=1 ones matmul only if bo != 0).

Scheduling: x/Wv are DMAed as column-half tiles ordered to feed the
kt-major v-projection waves (compute starts ~2us in, doubling as HAM
warmup); q/k projections for tile dt+1 are woven into tile dt's
attention; the first o-proj chains are woven into dt=7.
"""

import sys

if "/opt/trn_rl_repo" not in sys.path:
    sys.path.insert(0, "/opt/trn_rl_repo")

import numpy as np
import ml_dtypes

BF16 = ml_dtypes.bfloat16

N = 1024  # tokens
D = 1024  # model dim
H = 16    # heads
HD = 64   # head dim
P = 128   # partitions
F = 512   # psum free-dim tile
NCORES = 8
SCALING = 1.0 / 16.0  # lora alpha/rank
SCALE = HD ** -0.5

_CACHE = {}


def _build(with_bias):
    import concourse.bacc as bacc
    import concourse.mybir as mybir
    import concourse.tile as tile

    f32 = mybir.dt.float32
    f16 = mybir.dt.float16
    bf16 = mybir.dt.bfloat16
    Exp = mybir.ActivationFunctionType.Exp
    Copy = mybir.ActivationFunctionType.Copy

    nc = bacc.Bacc("TRN2", target_bir_lowering=False, debug=False)

    xT_e = nc.declare_dram_parameter("xT", [D, N], bf16, isOutput=False)
    wT_e = {
        nm: nc.declare_dram_parameter(nm, [D, D], bf16, isOutput=False)
        for nm in ("WqT", "WkT", "WvT", "WoT")
    }
    bo_e = nc.declare_dram_parameter("BoT", [1, D], bf16, isOutput=False)
    out_e = nc.declare_dram_parameter("out", [N, D], bf16, isOutput=True)

    with tile.TileContext(nc) as tc:
        with (
            tc.tile_pool(name="wpool", bufs=1) as wpool,
            tc.tile_pool(name="stage", bufs=2) as stage,
            tc.tile_pool(name="ps", bufs=1, space="PSUM") as ps,
        ):
            qs = [nc.sync, nc.scalar, nc.gpsimd]
            qi = 0

            def dma_tile(dst, src):
                nonlocal qi
                qs[qi % 3].dma_start(out=dst[:], in_=src)
                qi += 1

            # x and Wv live as column-half tiles [P, F]; DMA order is
            # chosen so each v-wave's inputs land just before it runs.
            xh = [[wpool.tile([P, F], bf16, tag=f"x_{t}_{h}",
                              name=f"x_{t}_{h}") for h in range(2)]
                  for t in range(8)]
            vh = [[wpool.tile([P, F], bf16, tag=f"Wv_{t}_{h}",
                              name=f"Wv_{t}_{h}") for h in range(2)]
                  for t in range(8)]
            T = {}
            for nm in ("Wq", "Wk", "Wo"):
                T[nm] = [wpool.tile([P, D], bf16, tag=f"T_{nm}_{t}",
                                    name=f"T_{nm}_{t}") for t in range(8)]
            for t in range(8):
                dma_tile(xh[t][0], xT_e[t * P:(t + 1) * P, 0:F])
                dma_tile(vh[t][0], wT_e["WvT"][t * P:(t + 1) * P, 0:F])
            for t in range(8):
                dma_tile(vh[t][1], wT_e["WvT"][t * P:(t + 1) * P, F:2 * F])
            for t in range(8):
                dma_tile(xh[t][1], xT_e[t * P:(t + 1) * P, F:2 * F])
            for nm in ("Wq", "Wk", "Wo"):
                for t in range(8):
                    dma_tile(T[nm][t], wT_e[nm + "T"][t * P:(t + 1) * P, :])
            bot = wpool.tile([1, D], bf16, tag="bot")
            dma_tile(bot, bo_e[:, :])

            def xcol(t, c0, c1):  # x[t] columns [c0, c1) across halves
                h = c0 // F
                return xh[t][h][:, c0 - h * F:c1 - h * F]

            onesh = wpool.tile([1, HD], f16, tag="onesh")
            nc.vector.memset(onesh[:], 1.0)
            onesb = wpool.tile([1, P], bf16, tag="onesb")
            nc.vector.memset(onesb[:], 1.0)

            # ---- tiny HAM kick, gated on the first x half-tile ----
            wps = ps.tile([P, F], f32, tag="projpsum", bufs=2)
            for _ in range(6):
                nc.tensor.matmul(wps[:, 0:256], xh[0][0][:, 0:P],
                                 xh[0][0][:, 0:256], start=True, stop=True)

            # ---- v natural, per-head layout [v_h | 1]; kt-major waves
            # gated on the half-tile DMA arrivals. ----
            VW = H * (HD + 1)  # 1040
            v_sb = [wpool.tile([P, VW], bf16, tag=f"v_{t}",
                               name=f"v_{t}") for t in range(8)]
            waves = [[(nt, 0) for nt in range(4)],
                     [(nt, 1) for nt in range(4)],
                     [(nt, 0) for nt in range(4, 8)],
                     [(nt, 1) for nt in range(4, 8)]]
            for wv, chains in enumerate(waves):
                pv = {}
                for ci, (nt, dh) in enumerate(chains):
                    if ci % 2 == 0:
                        pvt = ps.tile([P, 2 * F], f32, tag="spair", bufs=2,
                                      name=f"pvt_{wv}_{ci}")
                    pv[(nt, dh)] = pvt[:, (ci % 2) * F:(ci % 2 + 1) * F]
                for kt in range(8):
                    for (nt, dh) in chains:
                        nc.tensor.matmul(
                            pv[(nt, dh)], xcol(kt, nt * P, (nt + 1) * P),
                            vh[kt][dh][:],
                            start=(kt == 0), stop=(kt == 7))
                for (nt, dh) in chains:
                    vr = v_sb[nt][:].rearrange("p (h c) -> p h c", c=HD + 1)
                    pvr = pv[(nt, dh)].rearrange("p (h c) -> p h c", c=HD)
                    nc.vector.tensor_copy(
                        vr[:, dh * 8:(dh + 1) * 8, 0:HD], pvr[:])
                    if dh == 1:
                        nc.vector.memset(vr[:, :, HD:HD + 1], 1.0)

            # ---- q/k projection generator (dense only; lora folded) ----
            qks = {}

            def proj_gen(dt):
                qk = {}
                for nm, wnm in (("q", "Wq"), ("k", "Wk")):
                    dst = wpool.tile([P, D], bf16, tag=f"{nm}T",
                                     bufs=3, name=f"{nm}T_{dt}")
                    qk[nm] = dst
                    for nh in range(2):
                        ns = slice(nh * F, (nh + 1) * F)
                        pq = ps.tile([P, F], f32, tag="projpsum", bufs=2)
                        for kt in range(8):
                            nc.tensor.matmul(
                                pq[:], T[wnm][kt][:, dt * P:(dt + 1) * P],
                                xh[kt][nh][:],
                                start=(kt == 0), stop=(kt == 7))
                            yield
                        nc.vector.tensor_copy(dst[:, ns], pq[:])
                        yield
                qks[dt] = qk

            # proj(0) runs after the v-waves, paced by Wq/Wk arrivals
            for _ in proj_gen(0):
                pass

            # ---- output projection chain pieces ----
            attnT = [wpool.tile([P, D], bf16, tag=f"attnT_{t}",
                                name=f"attnT_{t}") for t in range(8)]

            def oproj_head(nt, dh, upto):
                pf = ps.tile([P, F], f32, tag="projpsum", bufs=2,
                             name=f"pf_{nt}_{dh}")
                for kt in range(upto):
                    nc.tensor.matmul(pf[:],
                                     attnT[kt][:, nt * P:(nt + 1) * P],
                                     T["Wo"][kt][:, dh * F:(dh + 1) * F],
                                     start=(kt == 0),
                                     stop=(kt == 7 and not with_bias))
                    yield pf

            def oproj_tail(pf, nt, dh, upto, on_act):
                ds = slice(dh * F, (dh + 1) * F)
                for kt in range(upto, 8):
                    nc.tensor.matmul(pf[:],
                                     attnT[kt][:, nt * P:(nt + 1) * P],
                                     T["Wo"][kt][:, ds],
                                     start=False,
                                     stop=(kt == 7 and not with_bias))
                if with_bias:
                    nc.tensor.matmul(pf[:], onesb[0:1, :], bot[0:1, ds],
                                     start=False, stop=True)
                osb = stage.tile([P, F], bf16, tag="osb", bufs=3)
                if on_act:
                    nc.scalar.activation(osb[:], pf[:], Copy)
                else:
                    nc.vector.tensor_copy(osb[:], pf[:])
                nc.sync.dma_start(out=out_e[nt * P:(nt + 1) * P, ds],
                                  in_=osb[:])

            def oproj_chain(nt, dh, on_act):
                pf = None
                for pf in oproj_head(nt, dh, 8):
                    yield
                oproj_tail(pf, nt, dh, 8, on_act)
                yield

            # ---- attention per dout-tile, two m-tiles per step with
            # PV lagged one step so it never waits on ACT ----
            woven = [(0, 0), (0, 1)]
            for dt in range(8):
                if dt < 7:
                    g = proj_gen(dt + 1)
                else:
                    def _dt7_gen():
                        for nt, dh in woven:
                            pfh = None
                            for pfh in oproj_head(nt, dh, 7):
                                yield
                            _pf_held.append((pfh, nt, dh))
                    _pf_held = []
                    g = _dt7_gen()
                h0 = 2 * dt
                qt = qks[dt]["q"]
                ktt = qks[dt]["k"]
                for nh in range(2):
                    ns = slice(nh * F, (nh + 1) * F)
                    po = {}
                    for h in (h0, h0 + 1):
                        po[h] = ps.tile([HD + 1, F], f32, tag="pvpsum",
                                        bufs=2, name=f"po_{h}_{nh}")
                    pte = {}

                    def qk_exp(mt):
                        spair = ps.tile([P, 2 * F], f32, tag="spair",
                                        bufs=2)
                        for hi, h in enumerate((h0, h0 + 1)):
                            ro = (h % 2) * HD
                            m0 = mt * P
                            nc.tensor.matmul(
                                spair[:, hi * F:(hi + 1) * F],
                                ktt[ro:ro + HD, m0:m0 + P],
                                qt[ro:ro + HD, ns], start=True, stop=True)
                        pte[mt] = stage.tile([P, 2 * F], bf16, tag="pt",
                                             bufs=4, name=f"pte_{mt}")
                        nc.scalar.activation(pte[mt][:], spair[:], Exp)

                    def pv(mt):
                        for hi, h in enumerate((h0, h0 + 1)):
                            nc.tensor.matmul(
                                po[h][:],
                                v_sb[mt][:, h * (HD + 1):(h + 1) * (HD + 1)],
                                pte[mt][:, hi * F:(hi + 1) * F],
                                start=(mt == 0), stop=(mt == 7))

                    for step in range(4):
                        qk_exp(2 * step)
                        qk_exp(2 * step + 1)
                        if step > 0:
                            pv(2 * step - 2)
                            pv(2 * step - 1)
                        for _ in range(3):
                            next(g, None)
                    pv(6)
                    pv(7)
                    # finalize both heads: DVE prep, batched broadcasts,
                    # then reciprocals + normalize into attnT
                    oah = {}
                    dn = {}
                    pbp = {}
                    for h in (h0, h0 + 1):
                        oah[h] = stage.tile([HD + 1, F], f32, tag="oah",
                                            bufs=3, name=f"oah_{h}")
                        nc.vector.tensor_copy(oah[h][:], po[h][:])
                        dn[h] = stage.tile([1, F], f16, tag="dn", bufs=3,
                                           name=f"dn_{h}")
                        nc.vector.tensor_copy(dn[h][:], oah[h][HD:HD + 1, :])
                    for h in (h0, h0 + 1):
                        pbp[h] = ps.tile([HD, F], f32, tag="pvpsum",
                                         bufs=2, name=f"pb_{h}")
                        nc.tensor.matmul(pbp[h][:], onesh[0:1, :], dn[h][:],
                                         start=True, stop=True)
                    for _ in range(2):
                        next(g, None)
                    for h in (h0, h0 + 1):
                        ro = (h % 2) * HD
                        pbs = stage.tile([HD, F], f32, tag="pbs", bufs=3,
                                         name=f"pbs_{h}")
                        nc.vector.reciprocal_approx_fast(pbs[:], pbp[h][:])
                        nc.vector.tensor_mul(attnT[dt][ro:ro + HD, ns],
                                             oah[h][0:HD, :], pbs[:])
                for _ in g:
                    pass

            # ---- finish woven o-proj chains, then the rest ----
            for i, (pfh, nt, dh) in enumerate(_pf_held):
                oproj_tail(pfh, nt, dh, 7, on_act=(i % 2 == 1))
            rest = [(nt, dh) for nt in range(8) for dh in range(2)
                    if (nt, dh) not in woven]
            for i, (nt, dh) in enumerate(rest):
                for _ in oproj_chain(nt, dh, on_act=(i % 2 == 1)):
                    pass
    nc.compile()
    return nc


def _get_nc(with_bias=False):
    key = ("nc", with_bias)
    if key not in _CACHE:
        _CACHE[key] = _build(with_bias)
    return _CACHE[key]


def _prep_shared(inputs):
    def fold(w, a, b, scl=1.0):
        w = np.asarray(w, np.float32)
        a = np.asarray(a, np.float32)
        b = np.asarray(b, np.float32)
        eff = (w + (b @ a) * SCALING) * scl
        return np.ascontiguousarray(eff.T.astype(BF16))

    shared = {
        "WqT": fold(inputs["Wq"], inputs["Aq"], inputs["Bq"], SCALE),
        "WkT": fold(inputs["Wk"], inputs["Ak"], inputs["Bk"]),
        "WvT": fold(inputs["Wv"], inputs["Av"], inputs["Bv"]),
        "WoT": fold(inputs["Wo"], inputs["Ao"], inputs["Bo"]),
        "BoT": np.ascontiguousarray(
            np.asarray(inputs["bo"], np.float32).reshape(1, D).astype(BF16)),
    }
    return shared


def kernel(**inputs):
    from concourse import bass_utils

    with_bias = bool(np.any(np.asarray(inputs["bo"], np.float32)))
    nc = _get_nc(with_bias)
    shared = _prep_shared(inputs)
    x = np.asarray(inputs["x"], np.float32)
    in_maps = []
    for i in range(NCORES):
        m = dict(shared)
        m["xT"] = np.ascontiguousarray(x[i].T.astype(BF16))
        in_maps.append(m)
    res = bass_utils.run_bass_kernel_spmd(nc, in_maps,
                                          core_ids=list(range(NCORES)))
    return np.stack([np.asarray(res.results[i]["out"]).astype(np.float32)
                     for i in range(NCORES)], axis=0)
